# revision 4
# baseline (speedup 1.0000x reference)
"""BiMamba block Trainium2 kernel (v3, t-half pipelined).

Sharding: 8 cores = (2 directions) x (4 batches); stage 1 runs one full Mamba
direction per core, stage 2 (second launch) combines directions via the
sigmoid gate and final projection, 8 cores = (4 batches) x (2 t-halves).

Stage 1 highlights:
- Zero GpSimd compute (Pool TT contends ~4.5x with DVE for SBUF ports).
- n-reduction y = sum_n h*C on the PE: identity-weight matmuls accumulate the
  16 states into PSUM; Dp*uc joins via a diag(Dp) matmul.
- dbub = (delta*u) x B built with broadcast-AP TTs; hc = h*C in place.
- The sequence is split into two 512-col halves: stage A (in_proj/conv/
  x_proj) of half 1 and the dm-half-1 out_proj pass of half 0 hide under the
  DVE-bound scan loops. Scans chain across halves via saved end states.
- Host prearranges all weight/input layouts for contiguous DMA descriptors.
"""

import numpy as np
import ml_dtypes

import concourse.bass as bass
from concourse import bacc
import concourse.tile as tile
import concourse.mybir as mybir
from concourse.bass_utils import run_bass_kernel_spmd

F32 = mybir.dt.float32
BF16 = mybir.dt.bfloat16
AF = mybir.ActivationFunctionType
OP = mybir.AluOpType
ts = bass.ts

D_MODEL = 1024
D_INNER = 2048
D_STATE = 16
D_CONV = 4
DT_RANK = 64
BATCH = 4
SEQ = 1024
TH = SEQ // 2

NDT = D_INNER // 128
NKT = D_MODEL // 128
NB = np.dtype(ml_dtypes.bfloat16)

G = 4
NG = D_STATE // G


def build_stage1():
    nc = bacc.Bacc("TRN2", target_bir_lowering=False, debug=False, num_devices=8)

    xt0_in = nc.dram_tensor("xt0", [128, NKT * TH], BF16, kind="ExternalInput")
    xt1_in = nc.dram_tensor("xt1", [128, NKT * TH], BF16, kind="ExternalInput")
    w1 = nc.dram_tensor("w1", [128, NDT * NKT * 128], BF16, kind="ExternalInput")
    w2 = nc.dram_tensor("w2", [128, NDT * NKT * 128], BF16, kind="ExternalInput")
    conv_w = nc.dram_tensor("conv_w", [128, NDT * D_CONV], F32, kind="ExternalInput")
    conv_b = nc.dram_tensor("conv_b", [128, NDT], F32, kind="ExternalInput")
    xproj = nc.dram_tensor("xproj", [128, NDT * 96], BF16, kind="ExternalInput")
    dt_w = nc.dram_tensor("dt_w", [DT_RANK, D_INNER], BF16, kind="ExternalInput")
    dt_b = nc.dram_tensor("dt_b", [128, NDT], F32, kind="ExternalInput")
    A_in = nc.dram_tensor("A", [128, NDT * D_STATE], F32, kind="ExternalInput")
    dpdiag = nc.dram_tensor("dpdiag", [128, NDT * 128], BF16, kind="ExternalInput")
    ident = nc.dram_tensor("ident", [128, 128], BF16, kind="ExternalInput")
    outproj = nc.dram_tensor("outproj", [D_INNER, D_MODEL], BF16, kind="ExternalInput")
    sel_in = nc.dram_tensor("sel", [96, 2 * D_STATE * 128], BF16, kind="ExternalInput")

    y_dir = nc.dram_tensor("y_dir", [D_MODEL, SEQ], BF16, kind="ExternalOutput")

    from contextlib import ExitStack

    with tile.TileContext(nc) as tc:
        with (
            tc.tile_pool(name="consts", bufs=1) as consts,
            tc.tile_pool(name="persist", bufs=1) as persist,
        ):
            cw = consts.tile([128, NDT * D_CONV], F32)
            nc.sync.dma_start(cw[:], conv_w[:])
            cb = consts.tile([128, NDT], F32)
            nc.sync.dma_start(cb[:], conv_b[:])
            dtb = consts.tile([128, NDT], F32)
            nc.sync.dma_start(dtb[:], dt_b[:])
            A_sb = consts.tile([128, NDT * D_STATE], F32)
            nc.sync.dma_start(A_sb[:], A_in[:])
            dpd_sb = consts.tile([128, NDT, 128], BF16)
            nc.sync.dma_start(dpd_sb[:], dpdiag[:])
            id_sb = consts.tile([128, 128], BF16)
            nc.sync.dma_start(id_sb[:], ident[:])
            dtw_sb = consts.tile([DT_RANK, D_INNER], BF16)
            nc.sync.dma_start(dtw_sb[:], dt_w[:])
            # xt halves as separate host-prearranged tensors: the t0 half
            # alone gates the first in_proj and gets the scalar queue to
            # itself; t1 follows on gpsimd
            xt_sb = persist.tile([128, 2, NKT, TH], BF16)
            half = NKT // 2 * TH
            nc.scalar.dma_start(
                xt_sb[:, 0, 0 : NKT // 2], xt0_in.ap()[:, 0:half]
            )
            nc.scalar.dma_start(
                xt_sb[:, 0, NKT // 2 : NKT], xt0_in.ap()[:, half : 2 * half]
            )
            nc.gpsimd.dma_start(xt_sb[:, 1], xt1_in[:])


            uch = [
                [persist.tile([128, TH], BF16, name=f"uc{h}_{d}") for d in range(NDT)]
                for h in range(2)
            ]
            dblh = [persist.tile([96, TH], BF16, name=f"dbl{h}") for h in range(2)]
            Bh = [
                [persist.tile([128, G, TH], BF16, name=f"b{h}_{q}") for q in range(NG)]
                for h in range(2)
            ]
            Ch = [
                [persist.tile([128, G, TH], BF16, name=f"c{h}_{q}") for q in range(NG)]
                for h in range(2)
            ]
            u_tail = persist.tile([128, NDT, 4], BF16)
            h_end = persist.tile([128, NDT * D_STATE], BF16)

            _es = ExitStack()
            wzp = _es.enter_context(tc.tile_pool(name="wz", bufs=2))
            dtp = _es.enter_context(tc.tile_pool(name="dt", bufs=2))
            dap = _es.enter_context(tc.tile_pool(name="dap", bufs=3))
            dbp = _es.enter_context(tc.tile_pool(name="dbp", bufs=2))
            hp = _es.enter_context(tc.tile_pool(name="hp", bufs=3))
            ops = _es.enter_context(tc.tile_pool(name="ops", bufs=2))
            obp = _es.enter_context(tc.tile_pool(name="obp", bufs=2))
            psC = _es.enter_context(tc.tile_pool(name="psC", bufs=1, space="PSUM"))
            psY = _es.enter_context(tc.tile_pool(name="psY", bufs=1, space="PSUM"))
            psD = _es.enter_context(tc.tile_pool(name="psD", bufs=1, space="PSUM"))
            _ea = ExitStack()
            wst = _ea.enter_context(tc.tile_pool(name="wst", bufs=2))
            stB = _ea.enter_context(tc.tile_pool(name="stB", bufs=1))
            cvt = _ea.enter_context(tc.tile_pool(name="cvt", bufs=2))
            psA = _ea.enter_context(tc.tile_pool(name="psA", bufs=1, space="PSUM"))
            psB = _ea.enter_context(tc.tile_pool(name="psB", bufs=1, space="PSUM"))
            xp_sb = stB.tile([128, NDT, 96], BF16)
            nc.scalar.dma_start(xp_sb[:], xproj[:])
            sel = stB.tile([96, 2 * D_STATE, 128], BF16)
            nc.scalar.dma_start(
                sel[:], sel_in.ap().rearrange("p (j m) -> p j m", m=128)
            )

            dbl_ps_h = {}

            def issueA(h, d):
                # in_proj u for half h, d-tile d
                eng = nc.sync if h == 0 else nc.gpsimd
                w1t = wst.tile([128, NKT, 128], BF16, tag="w1")
                eng.dma_start(w1t[:], w1.ap()[:, ts(d, NKT * 128)])
                u_sb = cvt.tile([128, 4 + TH], BF16, tag="u")
                if h == 0:
                    nc.vector.memset(u_sb[:, 0:4], 0.0)
                else:
                    nc.vector.tensor_copy(u_sb[:, 0:4], u_tail[:, d])
                ups = psA.tile([128, TH], F32, tag="ups")
                for k in range(NKT):
                    nc.tensor.matmul(
                        ups[:], w1t[:, k], xt_sb[:, h, k, :],
                        start=(k == 0), stop=(k == NKT - 1),
                    )
                nc.scalar.activation(u_sb[:, 4 : 4 + TH], ups[:], AF.Copy)
                if h == 0:
                    nc.vector.tensor_copy(u_tail[:, d], u_sb[:, TH : TH + 4])
                    # conv on DVE (prefix window, DVE otherwise idle)
                    p3 = cvt.tile([128, TH], BF16, tag="cv3")
                    nc.vector.tensor_scalar_mul(
                        p3[:], u_sb[:, 4 : 4 + TH], cw[:, d * 4 + 3 : d * 4 + 4]
                    )
                    p2 = cvt.tile([128, TH], BF16, tag="cv2")
                    nc.vector.scalar_tensor_tensor(
                        p2[:], u_sb[:, 3 : 3 + TH],
                        cw[:, d * 4 + 2 : d * 4 + 3], p3[:], OP.mult, OP.add,
                    )
                    p1 = cvt.tile([128, TH], BF16, tag="cv1")
                    nc.vector.scalar_tensor_tensor(
                        p1[:], u_sb[:, 2 : 2 + TH],
                        cw[:, d * 4 + 1 : d * 4 + 2], p2[:], OP.mult, OP.add,
                    )
                    p0 = cvt.tile([128, TH], BF16, tag="cv0")
                    nc.vector.scalar_tensor_tensor(
                        p0[:], u_sb[:, 1 : 1 + TH],
                        cw[:, d * 4 : d * 4 + 1], p1[:], OP.mult, OP.add,
                    )
                    nc.scalar.activation(
                        uch[h][d][:], p0[:], AF.Silu, bias=cb[:, d : d + 1]
                    )
                else:
                    # conv on PE: 4 accumulating diag matmuls over shifted u;
                    # diag(cw[:, d*4+k]) built transiently on Scalar
                    cdg = cvt.tile([128, D_CONV, 128], BF16, tag="cdg")
                    for k in range(D_CONV):
                        nc.scalar.activation(
                            cdg[:, k], id_sb[:], AF.Copy,
                            scale=cw[:, d * 4 + k : d * 4 + k + 1],
                        )
                    cps = psA.tile([128, TH], F32, tag="ups")
                    for k in range(D_CONV):
                        nc.tensor.matmul(
                            cps[:], cdg[:, k], u_sb[:, k + 1 : k + 1 + TH],
                            start=(k == 0), stop=(k == D_CONV - 1),
                        )
                    nc.scalar.activation(
                        uch[h][d][:], cps[:], AF.Silu, bias=cb[:, d : d + 1]
                    )
                nc.tensor.matmul(
                    dbl_ps_h[h][0:96, :], xp_sb[:, d], uch[h][d][:],
                    start=(d == 0), stop=(d == NDT - 1),
                )

            def issueBC(h):
                nc.vector.tensor_copy(dblh[h][:], dbl_ps_h[h][0:96, :])
                for q in range(NG):
                    for i in range(G):
                        n = q * G + i
                        for j, dest in ((n, Bh[h][q]), (D_STATE + n, Ch[h][q])):
                            bps = psA.tile([128, TH], F32, tag="ups")
                            nc.tensor.matmul(
                                bps[:], sel[:, j], dblh[h][:],
                                start=True, stop=True,
                            )
                            if j < D_STATE:
                                nc.vector.tensor_copy(dest[:, i, :], bps[:])
                            else:
                                nc.scalar.activation(dest[:, i, :], bps[:], AF.Copy)


            def issueC(h, d, op_ps):
                # dt_proj -> delta = softplus via exp/ln
                esb = dtp.tile([128, TH], F32, tag="esb", bufs=1)
                dps = psC.tile([128, TH], F32, tag="mm")
                nc.tensor.matmul(
                    dps[:], dtw_sb[:, ts(d, 128)], dblh[h][0:DT_RANK, :],
                    start=True, stop=True,
                )
                nc.scalar.activation(
                    esb[:], dps[:], AF.Exp, bias=dtb[:, d : d + 1]
                )
                delta = dtp.tile([128, TH], F32, tag="delta")
                nc.scalar.activation(delta[:], esb[:], AF.Ln, bias=1.0)
                dbu = dtp.tile([128, TH], BF16, tag="dbu")
                nc.vector.tensor_tensor(dbu[:], delta[:], uch[h][d][:], OP.mult)
                # z + silu
                wzt = wzp.tile([128, NKT, 128], BF16, tag="wz")
                nc.gpsimd.dma_start(wzt[:], w2.ap()[:, ts(d, NKT * 128)])
                zps = psC.tile([128, TH], F32, tag="mm")
                for k in range(NKT):
                    nc.tensor.matmul(
                        zps[:], wzt[:, k], xt_sb[:, h, k, :],
                        start=(k == 0), stop=(k == NKT - 1),
                    )
                sz = dtp.tile([128, TH], BF16, tag="sz")
                nc.scalar.activation(sz[:], zps[:], AF.Silu)
                # y accumulator
                psum_y = psY.tile([128, TH], F32, tag="py")
                nc.tensor.matmul(
                    psum_y[:], dpd_sb[:, d], uch[h][d][:],
                    start=True, stop=False,
                )
                for q in range(NG):
                    dA = dap.tile([128, G, TH], BF16, tag="dA")
                    for i in range(G):
                        n = G * q + i
                        nc.scalar.activation(
                            dA[:, i, :], delta[:], AF.Exp,
                            scale=A_sb[:, d * D_STATE + n : d * D_STATE + n + 1],
                        )
                    hes = h_end[:, d * D_STATE + q * G : d * D_STATE + (q + 1) * G]
                    if h == 1:
                        corr = dtp.tile([128, G], BF16, tag="corr")
                        nc.vector.tensor_tensor(
                            corr[:],
                            dA[:, :, 0:1].rearrange("p g one -> p (g one)"),
                            hes, OP.mult,
                        )
                    nc.vector.memset(dA[:, :, 0:1], 0.0)
                    dbub = dbp.tile([128, G, TH], BF16, tag="dbub")
                    nc.vector.tensor_tensor(
                        dbub[:], Bh[h][q][:],
                        dbu[:, None, :].broadcast_to((128, G, TH)),
                        OP.mult,
                    )
                    if h == 1:
                        db0 = dbub[:, :, 0:1].rearrange("p g one -> p (g one)")
                        nc.vector.tensor_tensor(db0, db0, corr[:], OP.add)
                    ht = hp.tile([128, G, TH], BF16, tag="h")
                    nc.vector.tensor_tensor_scan(
                        ht[:].rearrange("p g t -> p (g t)"),
                        dA[:].rearrange("p g t -> p (g t)"),
                        dbub[:].rearrange("p g t -> p (g t)"),
                        0.0, OP.mult, OP.add,
                    )
                    if h == 0:
                        nc.vector.tensor_copy(
                            hes,
                            ht[:, :, TH - 1 : TH].rearrange("p g one -> p (g one)"),
                        )
                    nc.vector.tensor_tensor(ht[:], ht[:], Ch[h][q][:], OP.mult)
                    for i in range(G):
                        nc.tensor.matmul(
                            psum_y[:], id_sb[:], ht[:, i, :],
                            start=False, stop=(q == NG - 1 and i == G - 1),
                        )
                yg = uch[h][d]
                nc.vector.tensor_tensor(yg[:], psum_y[:], sz[:], OP.mult)
                # out_proj dm-half 0, this t-half (in-loop accumulation)
                opw = ops.tile([128, 512], BF16, tag="opw")
                nc.sync.dma_start(opw[:], outproj.ap()[ts(d, 128), 0:512])
                for mi in range(4):
                    nc.tensor.matmul(
                        op_ps[mi][:], opw[:, ts(mi, 128)], yg[:],
                        start=(d == 0), stop=(d == NDT - 1),
                    )

            def writeback(op_ps, dmh, h, nmi=4, mi0=0):
                for mi in range(mi0, mi0 + nmi):
                    ob = obp.tile([128, TH], BF16, tag="ob")
                    nc.scalar.activation(ob[:], op_ps[mi - mi0][:], AF.Copy)
                    nc.scalar.dma_start(
                        y_dir.ap()[ts(dmh * 4 + mi, 128), h * TH : (h + 1) * TH],
                        ob[:],
                    )

            # ---------------- schedule ----------------
            dbl_ps_h[0] = psB.tile([128, TH], F32, tag="dbl", name="dblps0")
            for d in range(NDT):
                issueA(0, d)
            issueBC(0)
            dbl_ps_h[1] = psB.tile([128, TH], F32, tag="dbl", name="dblps1")

            op_ps_a = [
                psD.tile([128, TH], F32, tag=f"ip{mi}", name=f"pa{mi}")
                for mi in range(4)
            ]
            for d in range(NDT):
                issueC(0, d, op_ps_a)
                issueA(1, d)
            issueBC(1)
            writeback(op_ps_a, 0, 0)
            _ea.close()  # free stage-A pools (psA/psB banks)
            psT = _es.enter_context(tc.tile_pool(name="psT", bufs=1, space="PSUM"))

            op_ps_b = [
                psD.tile([128, TH], F32, tag=f"ip{mi}", name=f"pb{mi}")
                for mi in range(4)
            ]
            # dm-half 1 of t0: 2 groups hide in the t1 loop
            tp = [
                psT.tile([128, TH], F32, tag=f"tp{mi}", name=f"tp{mi}")
                for mi in range(2)
            ]
            for d in range(NDT):
                issueC(1, d, op_ps_b)
                opw2 = ops.tile([128, 256], BF16, tag="opw2")
                nc.sync.dma_start(opw2[:], outproj.ap()[ts(d, 128), 512:768])
                for mi in range(2):
                    nc.tensor.matmul(
                        tp[mi][:], opw2[:, ts(mi, 128)], uch[0][d][:],
                        start=(d == 0), stop=(d == NDT - 1),
                    )
            writeback(op_ps_b, 0, 1)
            writeback(tp, 1, 0, nmi=2, mi0=0)

            # exposed tail: dm-half 1 of t0 (mi 2,3) + dm-half 1 of t1 (4 mi)
            tp2 = [
                psT.tile([128, TH], F32, tag=f"tp{mi}", name=f"tq{mi}")
                for mi in range(2)
            ]
            op_ps_c = [
                psD.tile([128, TH], F32, tag=f"ip{mi}", name=f"pc{mi}")
                for mi in range(4)
            ]
            for d in range(NDT):
                opw3 = ops.tile([128, 256], BF16, tag="opw2")
                nc.sync.dma_start(opw3[:], outproj.ap()[ts(d, 128), 768:1024])
                for mi in range(2):
                    nc.tensor.matmul(
                        tp2[mi][:], opw3[:, ts(mi, 128)], uch[0][d][:],
                        start=(d == 0), stop=(d == NDT - 1),
                    )
                opw4 = ops.tile([128, 512], BF16, tag="opw")
                nc.scalar.dma_start(opw4[:], outproj.ap()[ts(d, 128), 512:1024])
                for mi in range(4):
                    nc.tensor.matmul(
                        op_ps_c[mi][:], opw4[:, ts(mi, 128)], uch[1][d][:],
                        start=(d == 0), stop=(d == NDT - 1),
                    )
            writeback(tp2, 1, 0, nmi=2, mi0=2)
            writeback(op_ps_c, 1, 1)
            _es.close()

    nc.compile()
    return nc




def build_stage2():
    nc = bacc.Bacc("TRN2", target_bir_lowering=False, debug=False, num_devices=8)

    TH = SEQ // 2
    yA = nc.dram_tensor("yA", [128, NKT * TH], BF16, kind="ExternalInput")
    yB = nc.dram_tensor("yB", [128, NKT * TH], BF16, kind="ExternalInput")
    gwA = nc.dram_tensor("gwA", [128, NKT * NKT * 128], BF16, kind="ExternalInput")
    gwB = nc.dram_tensor("gwB", [128, NKT * NKT * 128], BF16, kind="ExternalInput")
    gb = nc.dram_tensor("gb", [128, NKT], F32, kind="ExternalInput")
    pw = nc.dram_tensor("pw", [128, NKT * NKT * 128], BF16, kind="ExternalInput")
    pb = nc.dram_tensor("pb", [128, NKT], F32, kind="ExternalInput")

    out = nc.dram_tensor("out", [D_MODEL, TH], F32, kind="ExternalOutput")

    with tile.TileContext(nc) as tc:
        with (
            tc.tile_pool(name="sb", bufs=1) as sb,
            tc.tile_pool(name="tmp", bufs=3) as tmp,
            tc.tile_pool(name="ps", bufs=3, space="PSUM") as ps,
        ):
            gb_sb = sb.tile([128, NKT], F32)
            nc.sync.dma_start(gb_sb[:], gb[:])
            pb_sb = sb.tile([128, NKT], F32)
            nc.sync.dma_start(pb_sb[:], pb[:])
            gwa_sb = sb.tile([128, NKT, NKT, 128], BF16)
            gwb_sb = sb.tile([128, NKT, NKT, 128], BF16)
            pw_sb = sb.tile([128, NKT, NKT, 128], BF16)
            for m in range(NKT):
                nc.scalar.dma_start(
                    gwa_sb[:, m], gwA.ap()[:, ts(m, NKT * 128)]
                )
                nc.sync.dma_start(
                    gwb_sb[:, m], gwB.ap()[:, ts(m, NKT * 128)]
                )
            for m in range(NKT):
                nc.sync.dma_start(pw_sb[:, m], pw.ap()[:, ts(m, NKT * 128)])
            ya_sb = sb.tile([128, NKT, TH], BF16)
            nc.sync.dma_start(ya_sb[:], yA[:])
            yb_sb = sb.tile([128, NKT, TH], BF16)
            nc.sync.dma_start(yb_sb[:], yB[:])
            yc_sb = sb.tile([128, NKT, TH], BF16)
            for m in range(NKT):
                gps = ps.tile([128, TH], F32, tag="g")
                for k in range(NKT):
                    nc.tensor.matmul(
                        gps[:], gwa_sb[:, m, k], ya_sb[:, k, :],
                        start=(k == 0), stop=False,
                    )
                for k in range(NKT):
                    nc.tensor.matmul(
                        gps[:], gwb_sb[:, m, k], yb_sb[:, k, :],
                        start=False, stop=(k == NKT - 1),
                    )
                g = tmp.tile([128, TH], BF16, tag="gg")
                nc.scalar.activation(
                    g[:], gps[:], AF.Sigmoid, bias=gb_sb[:, m : m + 1]
                )
                # y = yB + g*(yA - yB)
                dsub = tmp.tile([128, TH], BF16, tag="dsub")
                nc.vector.tensor_tensor(
                    dsub[:], ya_sb[:, m, :], yb_sb[:, m, :], OP.subtract
                )
                gm = tmp.tile([128, TH], BF16, tag="gm")
                nc.vector.tensor_tensor(gm[:], g[:], dsub[:], OP.mult)
                nc.vector.tensor_tensor(
                    yc_sb[:, m, :], yb_sb[:, m, :], gm[:], OP.add
                )
            for m2 in range(NKT):
                pps = ps.tile([128, TH], F32, tag="p")
                for k in range(NKT):
                    nc.tensor.matmul(
                        pps[:], pw_sb[:, m2, k], yc_sb[:, k, :],
                        start=(k == 0), stop=(k == NKT - 1),
                    )
                ob = tmp.tile([128, TH], F32, tag="ob")
                nc.scalar.activation(
                    ob[:], pps[:], AF.Identity, bias=pb_sb[:, m2 : m2 + 1]
                )
                nc.sync.dma_start(out.ap()[ts(m2, 128)], ob[:])

    nc.compile()
    return nc




def _tile_vec(v, nt):
    return np.ascontiguousarray(np.asarray(v, np.float32).reshape(nt, 128).T)


def _col_tiles(w):
    """[K, M] -> [128, M//128, K//128, 128]: w2[p, m, k, f] = w[k*128+p, m*128+f]"""
    K, M = w.shape
    return np.ascontiguousarray(
        w.reshape(K // 128, 128, M // 128, 128).transpose(1, 2, 0, 3)
    )


_CACHE = {}


def kernel(**inputs):
    inputs = {k: np.asarray(v) for k, v in inputs.items()}
    if "s1" not in _CACHE:
        _CACHE["s1"] = build_stage1()
        _CACHE["s2"] = build_stage2()
    nc1, nc2 = _CACHE["s1"], _CACHE["s2"]

    x = inputs["x"].astype(np.float32)  # [B, L, D]

    sel_np = np.zeros((96, 2 * D_STATE, 128), np.float32)
    for j in range(2 * D_STATE):
        sel_np[64 + j, j, :] = 1.0
    sel_np = sel_np.reshape(96, 2 * D_STATE * 128).astype(NB)

    ident = np.eye(128, dtype=np.float32).astype(NB)

    maps1 = []
    for core in range(8):
        s = "f" if core < 4 else "b"
        b = core % 4
        xb = x[b]
        if s == "b":
            xb = xb[::-1]
        inproj = inputs[f"inproj_{s}"].astype(np.float32)
        wu = inproj[:, :D_INNER]
        wz = inproj[:, D_INNER:]
        # xt[p, kt, t] = x[t, kt*128+p], split into t-halves
        xt = np.ascontiguousarray(
            xb.T.reshape(NKT, 128, SEQ).transpose(1, 0, 2)
        )
        dp = np.asarray(inputs[f"Dp_{s}"], np.float32).reshape(NDT, 128)
        dpdiag = np.zeros((128, NDT, 128), np.float32)
        for dd in range(NDT):
            np.fill_diagonal(dpdiag[:, dd, :], dp[dd])
        xp = np.asarray(inputs[f"xproj_{s}"], np.float32)
        xp2 = np.ascontiguousarray(
            xp.reshape(NDT, 128, 96).transpose(1, 0, 2)
        ).reshape(128, NDT * 96)
        maps1.append(
            dict(
                xt0=np.ascontiguousarray(xt[:, :, :TH]).reshape(
                    128, NKT * TH
                ).astype(NB),
                xt1=np.ascontiguousarray(xt[:, :, TH:]).reshape(
                    128, NKT * TH
                ).astype(NB),
                w1=_col_tiles(wu).reshape(128, NDT * NKT * 128).astype(NB),
                w2=_col_tiles(wz).reshape(128, NDT * NKT * 128).astype(NB),
                conv_w=np.ascontiguousarray(
                    np.asarray(inputs[f"conv_w_{s}"], np.float32)
                    .reshape(NDT, 128, D_CONV)
                    .transpose(1, 0, 2)
                    .reshape(128, NDT * D_CONV)
                ),
                conv_b=_tile_vec(inputs[f"conv_b_{s}"], NDT),
                xproj=xp2.astype(NB),
                dt_w=inputs[f"dt_w_{s}"].astype(NB),
                dt_b=_tile_vec(inputs[f"dt_b_{s}"], NDT),
                A=np.ascontiguousarray(
                    (-np.exp(np.asarray(inputs[f"Alog_{s}"], np.float32)))
                    .reshape(NDT, 128, D_STATE)
                    .transpose(1, 0, 2)
                    .reshape(128, NDT * D_STATE)
                ),
                dpdiag=dpdiag.reshape(128, NDT * 128).astype(NB),
                ident=ident,
                outproj=inputs[f"outproj_{s}"].astype(NB),
                sel=sel_np,
            )
        )
    global _last_maps1
    _last_maps1 = maps1
    res1 = run_bass_kernel_spmd(nc1, maps1, list(range(8)))
    y_dirs = [res1.results[c]["y_dir"] for c in range(8)]
    for c in range(4, 8):
        y_dirs[c] = y_dirs[c][:, ::-1]

    gate_w = inputs["gate_w"].astype(np.float32)
    gwA = _col_tiles(gate_w[:D_MODEL]).reshape(128, NKT * NKT * 128).astype(NB)
    gwB = _col_tiles(gate_w[D_MODEL:]).reshape(128, NKT * NKT * 128).astype(NB)
    gb = _tile_vec(inputs["gate_b"], NKT)
    pw = _col_tiles(inputs["proj_w"].astype(np.float32)).reshape(
        128, NKT * NKT * 128
    ).astype(NB)
    pb = _tile_vec(inputs["proj_b"], NKT)

    def _ytile(y):
        return np.ascontiguousarray(
            y.reshape(NKT, 128, TH).transpose(1, 0, 2)
        ).reshape(128, NKT * TH)

    maps2 = []
    for core in range(8):
        b = core % 4
        half = core // 4
        sl = slice(half * TH, (half + 1) * TH)
        maps2.append(
            dict(
                yA=_ytile(np.ascontiguousarray(y_dirs[b][:, sl])),
                yB=_ytile(np.ascontiguousarray(y_dirs[4 + b][:, sl])),
                gwA=gwA, gwB=gwB, gb=gb, pw=pw, pb=pb,
            )
        )
    global _last_maps2
    _last_maps2 = maps2
    res2 = run_bass_kernel_spmd(nc2, maps2, list(range(8)))

    out = np.empty((BATCH, SEQ, D_MODEL), np.float32)
    for core in range(8):
        b = core % 4
        half = core // 4
        o = res2.results[core]["out"]
        out[b, half * TH : (half + 1) * TH, :] = o.T
    return out


# revision 11
# speedup vs baseline: 1.0054x; 1.0054x over previous
"""BiMamba block Trainium2 kernel (v3, t-half pipelined).

Sharding: 8 cores = (2 directions) x (4 batches); stage 1 runs one full Mamba
direction per core, stage 2 (second launch) combines directions via the
sigmoid gate and final projection, 8 cores = (4 batches) x (2 t-halves).

Stage 1 highlights:
- Zero GpSimd compute (Pool TT contends ~4.5x with DVE for SBUF ports).
- n-reduction y = sum_n h*C on the PE: identity-weight matmuls accumulate the
  16 states into PSUM; Dp*uc joins via a diag(Dp) matmul.
- dbub = (delta*u) x B built with broadcast-AP TTs; hc = h*C in place.
- The sequence is split into two 512-col halves: stage A (in_proj/conv/
  x_proj) of half 1 and the dm-half-1 out_proj pass of half 0 hide under the
  DVE-bound scan loops. Scans chain across halves via saved end states.
- Host prearranges all weight/input layouts for contiguous DMA descriptors.
"""

import numpy as np
import ml_dtypes

import concourse.bass as bass
from concourse import bacc
import concourse.tile as tile
import concourse.mybir as mybir
from concourse.bass_utils import run_bass_kernel_spmd

F32 = mybir.dt.float32
BF16 = mybir.dt.bfloat16
AF = mybir.ActivationFunctionType
OP = mybir.AluOpType
ts = bass.ts

D_MODEL = 1024
D_INNER = 2048
D_STATE = 16
D_CONV = 4
DT_RANK = 64
BATCH = 4
SEQ = 1024
TH = SEQ // 2

NDT = D_INNER // 128
NKT = D_MODEL // 128
NB = np.dtype(ml_dtypes.bfloat16)

G = 4
NG = D_STATE // G


def build_stage1():
    nc = bacc.Bacc("TRN2", target_bir_lowering=False, debug=False, num_devices=8)

    xt0_in = nc.dram_tensor("xt0", [128, NKT * TH], BF16, kind="ExternalInput")
    xt1_in = nc.dram_tensor("xt1", [128, NKT * TH], BF16, kind="ExternalInput")
    w1 = nc.dram_tensor("w1", [128, NDT * NKT * 128], BF16, kind="ExternalInput")
    w2 = nc.dram_tensor("w2", [128, NDT * NKT * 128], BF16, kind="ExternalInput")
    conv_w = nc.dram_tensor("conv_w", [128, NDT * D_CONV], F32, kind="ExternalInput")
    conv_b = nc.dram_tensor("conv_b", [128, NDT], F32, kind="ExternalInput")
    xproj = nc.dram_tensor("xproj", [128, NDT * 96], BF16, kind="ExternalInput")
    dt_w = nc.dram_tensor("dt_w", [DT_RANK, D_INNER], BF16, kind="ExternalInput")
    dt_b = nc.dram_tensor("dt_b", [128, NDT], F32, kind="ExternalInput")
    A_in = nc.dram_tensor("A", [128, NDT * D_STATE], F32, kind="ExternalInput")
    dpdiag = nc.dram_tensor("dpdiag", [128, NDT * 128], BF16, kind="ExternalInput")
    ident = nc.dram_tensor("ident", [128, 128], BF16, kind="ExternalInput")
    outproj = nc.dram_tensor("outproj", [D_INNER, D_MODEL], BF16, kind="ExternalInput")
    sel_in = nc.dram_tensor("sel", [96, 2 * D_STATE * 128], BF16, kind="ExternalInput")

    y_dir = nc.dram_tensor("y_dir", [D_MODEL, SEQ], BF16, kind="ExternalOutput")

    from contextlib import ExitStack

    with tile.TileContext(nc) as tc:
        with (
            tc.tile_pool(name="consts", bufs=1) as consts,
            tc.tile_pool(name="persist", bufs=1) as persist,
        ):
            cw = consts.tile([128, NDT * D_CONV], F32)
            nc.sync.dma_start(cw[:], conv_w[:])
            cb = consts.tile([128, NDT], F32)
            nc.sync.dma_start(cb[:], conv_b[:])
            dtb = consts.tile([128, NDT], F32)
            nc.sync.dma_start(dtb[:], dt_b[:])
            A_sb = consts.tile([128, NDT * D_STATE], F32)
            nc.sync.dma_start(A_sb[:], A_in[:])
            dpd_sb = consts.tile([128, NDT, 128], BF16)
            nc.sync.dma_start(dpd_sb[:], dpdiag[:])
            id_sb = consts.tile([128, 128], BF16)
            nc.sync.dma_start(id_sb[:], ident[:])
            dtw_sb = consts.tile([DT_RANK, D_INNER], BF16)
            nc.sync.dma_start(dtw_sb[:], dt_w[:])
            # xt halves as separate host-prearranged tensors: the t0 half
            # alone gates the first in_proj and gets the scalar queue to
            # itself; t1 follows on gpsimd
            xt_sb = persist.tile([128, 2, NKT, TH], BF16)
            piece = 2 * TH
            for pi in range(4):
                nc.scalar.dma_start(
                    xt_sb[:, 0, 2 * pi : 2 * pi + 2],
                    xt0_in.ap()[:, pi * piece : (pi + 1) * piece],
                )
            nc.gpsimd.dma_start(xt_sb[:, 1], xt1_in[:])


            uch = [
                [persist.tile([128, TH], BF16, name=f"uc{h}_{d}") for d in range(NDT)]
                for h in range(2)
            ]
            dblh = [persist.tile([96, TH], BF16, name=f"dbl{h}") for h in range(2)]
            Bh = [
                [persist.tile([128, G, TH], BF16, name=f"b{h}_{q}") for q in range(NG)]
                for h in range(2)
            ]
            Ch = [
                [persist.tile([128, G, TH], BF16, name=f"c{h}_{q}") for q in range(NG)]
                for h in range(2)
            ]
            u_tail = persist.tile([128, NDT, 4], BF16)
            h_end = persist.tile([128, NDT * D_STATE], BF16)

            _es = ExitStack()
            wzp = _es.enter_context(tc.tile_pool(name="wz", bufs=2))
            dtp = _es.enter_context(tc.tile_pool(name="dt", bufs=2))
            dap = _es.enter_context(tc.tile_pool(name="dap", bufs=3))
            dbp = _es.enter_context(tc.tile_pool(name="dbp", bufs=2))
            hp = _es.enter_context(tc.tile_pool(name="hp", bufs=3))
            ops = _es.enter_context(tc.tile_pool(name="ops", bufs=2))
            obp = _es.enter_context(tc.tile_pool(name="obp", bufs=2))
            psC = _es.enter_context(tc.tile_pool(name="psC", bufs=1, space="PSUM"))
            psY = _es.enter_context(tc.tile_pool(name="psY", bufs=1, space="PSUM"))
            psD = _es.enter_context(tc.tile_pool(name="psD", bufs=1, space="PSUM"))
            _ea = ExitStack()
            wst = _ea.enter_context(tc.tile_pool(name="wst", bufs=2))
            stB = _ea.enter_context(tc.tile_pool(name="stB", bufs=1))
            cvt = _ea.enter_context(tc.tile_pool(name="cvt", bufs=2))
            psA = _ea.enter_context(tc.tile_pool(name="psA", bufs=1, space="PSUM"))
            psB = _ea.enter_context(tc.tile_pool(name="psB", bufs=1, space="PSUM"))
            xp_sb = stB.tile([128, NDT, 96], BF16)
            nc.scalar.dma_start(xp_sb[:], xproj[:])
            sel = stB.tile([96, 2 * D_STATE, 128], BF16)
            nc.scalar.dma_start(
                sel[:], sel_in.ap().rearrange("p (j m) -> p j m", m=128)
            )

            dbl_ps_h = {}

            def issueA(h, d):
                # in_proj u for half h, d-tile d
                eng = nc.sync if h == 0 else nc.gpsimd
                w1t = wst.tile([128, NKT, 128], BF16, tag="w1")
                eng.dma_start(w1t[:], w1.ap()[:, ts(d, NKT * 128)])
                u_sb = cvt.tile([128, 4 + TH], BF16, tag="u")
                if h == 0:
                    nc.vector.memset(u_sb[:, 0:4], 0.0)
                else:
                    nc.vector.tensor_copy(u_sb[:, 0:4], u_tail[:, d])
                ups = psA.tile([128, TH], F32, tag="ups")
                for k in range(NKT):
                    nc.tensor.matmul(
                        ups[:], w1t[:, k], xt_sb[:, h, k, :],
                        start=(k == 0), stop=(k == NKT - 1),
                    )
                nc.scalar.activation(u_sb[:, 4 : 4 + TH], ups[:], AF.Copy)
                if h == 0:
                    nc.vector.tensor_copy(u_tail[:, d], u_sb[:, TH : TH + 4])
                    # conv on DVE (prefix window, DVE otherwise idle)
                    p3 = cvt.tile([128, TH], BF16, tag="cv3")
                    nc.vector.tensor_scalar_mul(
                        p3[:], u_sb[:, 4 : 4 + TH], cw[:, d * 4 + 3 : d * 4 + 4]
                    )
                    p2 = cvt.tile([128, TH], BF16, tag="cv2")
                    nc.vector.scalar_tensor_tensor(
                        p2[:], u_sb[:, 3 : 3 + TH],
                        cw[:, d * 4 + 2 : d * 4 + 3], p3[:], OP.mult, OP.add,
                    )
                    p1 = cvt.tile([128, TH], BF16, tag="cv1")
                    nc.vector.scalar_tensor_tensor(
                        p1[:], u_sb[:, 2 : 2 + TH],
                        cw[:, d * 4 + 1 : d * 4 + 2], p2[:], OP.mult, OP.add,
                    )
                    p0 = cvt.tile([128, TH], BF16, tag="cv0")
                    nc.vector.scalar_tensor_tensor(
                        p0[:], u_sb[:, 1 : 1 + TH],
                        cw[:, d * 4 : d * 4 + 1], p1[:], OP.mult, OP.add,
                    )
                    nc.scalar.activation(
                        uch[h][d][:], p0[:], AF.Silu, bias=cb[:, d : d + 1]
                    )
                else:
                    # conv on PE: 4 accumulating diag matmuls over shifted u;
                    # diag(cw[:, d*4+k]) built transiently on Scalar
                    cdg = cvt.tile([128, D_CONV, 128], BF16, tag="cdg")
                    for k in range(D_CONV):
                        nc.scalar.activation(
                            cdg[:, k], id_sb[:], AF.Copy,
                            scale=cw[:, d * 4 + k : d * 4 + k + 1],
                        )
                    cps = psA.tile([128, TH], F32, tag="ups")
                    for k in range(D_CONV):
                        nc.tensor.matmul(
                            cps[:], cdg[:, k], u_sb[:, k + 1 : k + 1 + TH],
                            start=(k == 0), stop=(k == D_CONV - 1),
                        )
                    nc.scalar.activation(
                        uch[h][d][:], cps[:], AF.Silu, bias=cb[:, d : d + 1]
                    )
                nc.tensor.matmul(
                    dbl_ps_h[h][0:96, :], xp_sb[:, d], uch[h][d][:],
                    start=(d == 0), stop=(d == NDT - 1),
                )

            def issueBC(h):
                nc.vector.tensor_copy(dblh[h][:], dbl_ps_h[h][0:96, :])
                for q in range(NG):
                    for i in range(G):
                        n = q * G + i
                        for j, dest in ((n, Bh[h][q]), (D_STATE + n, Ch[h][q])):
                            bps = psA.tile([128, TH], F32, tag="ups")
                            nc.tensor.matmul(
                                bps[:], sel[:, j], dblh[h][:],
                                start=True, stop=True,
                            )
                            if j < D_STATE:
                                nc.vector.tensor_copy(dest[:, i, :], bps[:])
                            else:
                                nc.scalar.activation(dest[:, i, :], bps[:], AF.Copy)


            _heads = {}

            def issueHead(h, d):
                # dt_proj -> delta = softplus via exp/ln
                esb = dtp.tile([128, TH], F32, tag="esb", bufs=1)
                dps = psC.tile([128, TH], F32, tag="mm")
                nc.tensor.matmul(
                    dps[:], dtw_sb[:, ts(d, 128)], dblh[h][0:DT_RANK, :],
                    start=True, stop=True,
                )
                nc.scalar.activation(
                    esb[:], dps[:], AF.Exp, bias=dtb[:, d : d + 1]
                )
                delta = dtp.tile([128, TH], F32, tag="delta")
                nc.scalar.activation(delta[:], esb[:], AF.Ln, bias=1.0)
                dbu = dtp.tile([128, TH], BF16, tag="dbu")
                nc.vector.tensor_tensor(dbu[:], delta[:], uch[h][d][:], OP.mult)
                _heads[(h, d)] = (delta, dbu)

            def issueC(h, d, op_ps, mid=None):
                delta, dbu = _heads.pop((h, d))
                # z + silu
                wzt = wzp.tile([128, NKT, 128], BF16, tag="wz")
                nc.gpsimd.dma_start(wzt[:], w2.ap()[:, ts(d, NKT * 128)])
                zps = psC.tile([128, TH], F32, tag="mm")
                for k in range(NKT):
                    nc.tensor.matmul(
                        zps[:], wzt[:, k], xt_sb[:, h, k, :],
                        start=(k == 0), stop=(k == NKT - 1),
                    )
                sz = dtp.tile([128, TH], BF16, tag="sz")
                nc.scalar.activation(sz[:], zps[:], AF.Silu)
                if mid is not None:
                    mid()  # interleaved stage-A work: its Silu lands next to
                    # the z Silu, so the following dA Exps need one table load
                # y accumulator
                psum_y = psY.tile([128, TH], F32, tag="py")
                nc.tensor.matmul(
                    psum_y[:], dpd_sb[:, d], uch[h][d][:],
                    start=True, stop=False,
                )
                for q in range(NG):
                    dA = dap.tile([128, G, TH], BF16, tag="dA")
                    for i in range(G):
                        n = G * q + i
                        nc.scalar.activation(
                            dA[:, i, :], delta[:], AF.Exp,
                            scale=A_sb[:, d * D_STATE + n : d * D_STATE + n + 1],
                        )
                    hes = h_end[:, d * D_STATE + q * G : d * D_STATE + (q + 1) * G]
                    if h == 1:
                        corr = dtp.tile([128, G], BF16, tag="corr")
                        nc.vector.tensor_tensor(
                            corr[:],
                            dA[:, :, 0:1].rearrange("p g one -> p (g one)"),
                            hes, OP.mult,
                        )
                    nc.vector.memset(dA[:, :, 0:1], 0.0)
                    dbub = dbp.tile([128, G, TH], BF16, tag="dbub")
                    nc.vector.tensor_tensor(
                        dbub[:], Bh[h][q][:],
                        dbu[:, None, :].broadcast_to((128, G, TH)),
                        OP.mult,
                    )
                    if h == 1:
                        db0 = dbub[:, :, 0:1].rearrange("p g one -> p (g one)")
                        nc.vector.tensor_tensor(db0, db0, corr[:], OP.add)
                    ht = hp.tile([128, G, TH], BF16, tag="h")
                    nc.vector.tensor_tensor_scan(
                        ht[:].rearrange("p g t -> p (g t)"),
                        dA[:].rearrange("p g t -> p (g t)"),
                        dbub[:].rearrange("p g t -> p (g t)"),
                        0.0, OP.mult, OP.add,
                    )
                    if h == 0:
                        nc.vector.tensor_copy(
                            hes,
                            ht[:, :, TH - 1 : TH].rearrange("p g one -> p (g one)"),
                        )
                    nc.vector.tensor_tensor(ht[:], ht[:], Ch[h][q][:], OP.mult)
                    for i in range(G):
                        nc.tensor.matmul(
                            psum_y[:], id_sb[:], ht[:, i, :],
                            start=False, stop=(q == NG - 1 and i == G - 1),
                        )
                yg = uch[h][d]
                nc.vector.tensor_tensor(yg[:], psum_y[:], sz[:], OP.mult)
                # out_proj dm-half 0, this t-half (in-loop accumulation)
                opw = ops.tile([128, 512], BF16, tag="opw")
                nc.sync.dma_start(opw[:], outproj.ap()[ts(d, 128), 0:512])
                for mi in range(4):
                    nc.tensor.matmul(
                        op_ps[mi][:], opw[:, ts(mi, 128)], yg[:],
                        start=(d == 0), stop=(d == NDT - 1),
                    )

            def writeback(op_ps, dmh, h, nmi=4, mi0=0):
                for mi in range(mi0, mi0 + nmi):
                    ob = obp.tile([128, TH], BF16, tag="ob")
                    nc.scalar.activation(ob[:], op_ps[mi - mi0][:], AF.Copy)
                    nc.scalar.dma_start(
                        y_dir.ap()[ts(dmh * 4 + mi, 128), h * TH : (h + 1) * TH],
                        ob[:],
                    )

            # ---------------- schedule ----------------
            dbl_ps_h[0] = psB.tile([128, TH], F32, tag="dbl", name="dblps0")
            for d in range(NDT):
                issueA(0, d)
            issueBC(0)
            dbl_ps_h[1] = psB.tile([128, TH], F32, tag="dbl", name="dblps1")

            op_ps_a = [
                psD.tile([128, TH], F32, tag=f"ip{mi}", name=f"pa{mi}")
                for mi in range(4)
            ]
            issueHead(0, 0)
            for d in range(NDT):
                issueC(0, d, op_ps_a, mid=lambda d=d: issueA(1, d))
                if d + 1 < NDT:
                    issueHead(0, d + 1)
            issueBC(1)
            writeback(op_ps_a, 0, 0)
            _ea.close()  # free stage-A pools (psA/psB banks)
            psT = _es.enter_context(tc.tile_pool(name="psT", bufs=1, space="PSUM"))

            op_ps_b = [
                psD.tile([128, TH], F32, tag=f"ip{mi}", name=f"pb{mi}")
                for mi in range(4)
            ]
            # dm-half 1 of t0: 2 groups hide in the t1 loop
            tp = [
                psT.tile([128, TH], F32, tag=f"tp{mi}", name=f"tp{mi}")
                for mi in range(2)
            ]
            issueHead(1, 0)
            for d in range(NDT):
                issueC(1, d, op_ps_b)
                if d + 1 < NDT:
                    issueHead(1, d + 1)
                opw2 = ops.tile([128, 256], BF16, tag="opw2")
                nc.sync.dma_start(opw2[:], outproj.ap()[ts(d, 128), 512:768])
                for mi in range(2):
                    nc.tensor.matmul(
                        tp[mi][:], opw2[:, ts(mi, 128)], uch[0][d][:],
                        start=(d == 0), stop=(d == NDT - 1),
                    )
            writeback(op_ps_b, 0, 1)
            writeback(tp, 1, 0, nmi=2, mi0=0)

            # exposed tail: dm-half 1 of t0 (mi 2,3) + dm-half 1 of t1 (4 mi)
            tp2 = [
                psT.tile([128, TH], F32, tag=f"tp{mi}", name=f"tq{mi}")
                for mi in range(2)
            ]
            op_ps_c = [
                psD.tile([128, TH], F32, tag=f"ip{mi}", name=f"pc{mi}")
                for mi in range(4)
            ]
            for d in range(NDT):
                opw3 = ops.tile([128, 256], BF16, tag="opw2")
                nc.sync.dma_start(opw3[:], outproj.ap()[ts(d, 128), 768:1024])
                for mi in range(2):
                    nc.tensor.matmul(
                        tp2[mi][:], opw3[:, ts(mi, 128)], uch[0][d][:],
                        start=(d == 0), stop=(d == NDT - 1),
                    )
                opw4 = ops.tile([128, 512], BF16, tag="opw")
                nc.scalar.dma_start(opw4[:], outproj.ap()[ts(d, 128), 512:1024])
                for mi in range(4):
                    nc.tensor.matmul(
                        op_ps_c[mi][:], opw4[:, ts(mi, 128)], uch[1][d][:],
                        start=(d == 0), stop=(d == NDT - 1),
                    )
            writeback(tp2, 1, 0, nmi=2, mi0=2)
            writeback(op_ps_c, 1, 1)
            _es.close()

    nc.compile()
    return nc




def build_stage2():
    nc = bacc.Bacc("TRN2", target_bir_lowering=False, debug=False, num_devices=8)

    TH = SEQ // 2
    yA = nc.dram_tensor("yA", [128, NKT * TH], BF16, kind="ExternalInput")
    yB = nc.dram_tensor("yB", [128, NKT * TH], BF16, kind="ExternalInput")
    gwA = nc.dram_tensor("gwA", [128, NKT * NKT * 128], BF16, kind="ExternalInput")
    gwB = nc.dram_tensor("gwB", [128, NKT * NKT * 128], BF16, kind="ExternalInput")
    gb = nc.dram_tensor("gb", [128, NKT], F32, kind="ExternalInput")
    pw = nc.dram_tensor("pw", [128, NKT * NKT * 128], BF16, kind="ExternalInput")
    pb = nc.dram_tensor("pb", [128, NKT], F32, kind="ExternalInput")

    out = nc.dram_tensor("out", [D_MODEL, TH], F32, kind="ExternalOutput")

    with tile.TileContext(nc) as tc:
        with (
            tc.tile_pool(name="sb", bufs=1) as sb,
            tc.tile_pool(name="tmp", bufs=3) as tmp,
            tc.tile_pool(name="ps", bufs=3, space="PSUM") as ps,
        ):
            gb_sb = sb.tile([128, NKT], F32)
            nc.sync.dma_start(gb_sb[:], gb[:])
            pb_sb = sb.tile([128, NKT], F32)
            nc.sync.dma_start(pb_sb[:], pb[:])
            gwa_sb = sb.tile([128, NKT, NKT, 128], BF16)
            nc.scalar.dma_start(gwa_sb[:], gwA[:])
            gwb_sb = sb.tile([128, NKT, NKT, 128], BF16)
            nc.scalar.dma_start(gwb_sb[:], gwB[:])
            pw_sb = sb.tile([128, NKT, NKT, 128], BF16)
            nc.sync.dma_start(pw_sb[:], pw[:])
            ya_sb = sb.tile([128, NKT, TH], BF16)
            nc.sync.dma_start(ya_sb[:], yA[:])
            yb_sb = sb.tile([128, NKT, TH], BF16)
            nc.sync.dma_start(yb_sb[:], yB[:])
            yc_sb = sb.tile([128, NKT, TH], BF16)
            for m in range(NKT):
                gps = ps.tile([128, TH], F32, tag="g")
                for k in range(NKT):
                    nc.tensor.matmul(
                        gps[:], gwa_sb[:, m, k], ya_sb[:, k, :],
                        start=(k == 0), stop=False,
                    )
                for k in range(NKT):
                    nc.tensor.matmul(
                        gps[:], gwb_sb[:, m, k], yb_sb[:, k, :],
                        start=False, stop=(k == NKT - 1),
                    )
                g = tmp.tile([128, TH], BF16, tag="gg")
                nc.scalar.activation(
                    g[:], gps[:], AF.Sigmoid, bias=gb_sb[:, m : m + 1]
                )
                # y = yB + g*(yA - yB)
                dsub = tmp.tile([128, TH], BF16, tag="dsub")
                nc.vector.tensor_tensor(
                    dsub[:], ya_sb[:, m, :], yb_sb[:, m, :], OP.subtract
                )
                gm = tmp.tile([128, TH], BF16, tag="gm")
                nc.vector.tensor_tensor(gm[:], g[:], dsub[:], OP.mult)
                nc.vector.tensor_tensor(
                    yc_sb[:, m, :], yb_sb[:, m, :], gm[:], OP.add
                )
            for m2 in range(NKT):
                pps = ps.tile([128, TH], F32, tag="p")
                for k in range(NKT):
                    nc.tensor.matmul(
                        pps[:], pw_sb[:, m2, k], yc_sb[:, k, :],
                        start=(k == 0), stop=(k == NKT - 1),
                    )
                ob = tmp.tile([128, TH], F32, tag="ob")
                nc.scalar.activation(
                    ob[:], pps[:], AF.Identity, bias=pb_sb[:, m2 : m2 + 1]
                )
                nc.sync.dma_start(out.ap()[ts(m2, 128)], ob[:])

    nc.compile()
    return nc




def _tile_vec(v, nt):
    return np.ascontiguousarray(np.asarray(v, np.float32).reshape(nt, 128).T)


def _col_tiles(w):
    """[K, M] -> [128, M//128, K//128, 128]: w2[p, m, k, f] = w[k*128+p, m*128+f]"""
    K, M = w.shape
    return np.ascontiguousarray(
        w.reshape(K // 128, 128, M // 128, 128).transpose(1, 2, 0, 3)
    )


_CACHE = {}


def kernel(**inputs):
    inputs = {k: np.asarray(v) for k, v in inputs.items()}
    if "s1" not in _CACHE:
        _CACHE["s1"] = build_stage1()
        _CACHE["s2"] = build_stage2()
    nc1, nc2 = _CACHE["s1"], _CACHE["s2"]

    x = inputs["x"].astype(np.float32)  # [B, L, D]

    sel_np = np.zeros((96, 2 * D_STATE, 128), np.float32)
    for j in range(2 * D_STATE):
        sel_np[64 + j, j, :] = 1.0
    sel_np = sel_np.reshape(96, 2 * D_STATE * 128).astype(NB)

    ident = np.eye(128, dtype=np.float32).astype(NB)

    maps1 = []
    for core in range(8):
        s = "f" if core < 4 else "b"
        b = core % 4
        xb = x[b]
        if s == "b":
            xb = xb[::-1]
        inproj = inputs[f"inproj_{s}"].astype(np.float32)
        wu = inproj[:, :D_INNER]
        wz = inproj[:, D_INNER:]
        # xt[p, kt, t] = x[t, kt*128+p], split into t-halves
        xt = np.ascontiguousarray(
            xb.T.reshape(NKT, 128, SEQ).transpose(1, 0, 2)
        )
        dp = np.asarray(inputs[f"Dp_{s}"], np.float32).reshape(NDT, 128)
        dpdiag = np.zeros((128, NDT, 128), np.float32)
        for dd in range(NDT):
            np.fill_diagonal(dpdiag[:, dd, :], dp[dd])
        xp = np.asarray(inputs[f"xproj_{s}"], np.float32)
        xp2 = np.ascontiguousarray(
            xp.reshape(NDT, 128, 96).transpose(1, 0, 2)
        ).reshape(128, NDT * 96)
        maps1.append(
            dict(
                xt0=np.ascontiguousarray(xt[:, :, :TH]).reshape(
                    128, NKT * TH
                ).astype(NB),
                xt1=np.ascontiguousarray(xt[:, :, TH:]).reshape(
                    128, NKT * TH
                ).astype(NB),
                w1=_col_tiles(wu).reshape(128, NDT * NKT * 128).astype(NB),
                w2=_col_tiles(wz).reshape(128, NDT * NKT * 128).astype(NB),
                conv_w=np.ascontiguousarray(
                    np.asarray(inputs[f"conv_w_{s}"], np.float32)
                    .reshape(NDT, 128, D_CONV)
                    .transpose(1, 0, 2)
                    .reshape(128, NDT * D_CONV)
                ),
                conv_b=_tile_vec(inputs[f"conv_b_{s}"], NDT),
                xproj=xp2.astype(NB),
                dt_w=inputs[f"dt_w_{s}"].astype(NB),
                dt_b=_tile_vec(inputs[f"dt_b_{s}"], NDT),
                A=np.ascontiguousarray(
                    (-np.exp(np.asarray(inputs[f"Alog_{s}"], np.float32)))
                    .reshape(NDT, 128, D_STATE)
                    .transpose(1, 0, 2)
                    .reshape(128, NDT * D_STATE)
                ),
                dpdiag=dpdiag.reshape(128, NDT * 128).astype(NB),
                ident=ident,
                outproj=inputs[f"outproj_{s}"].astype(NB),
                sel=sel_np,
            )
        )
    global _last_maps1
    _last_maps1 = maps1
    res1 = run_bass_kernel_spmd(nc1, maps1, list(range(8)))
    y_dirs = [res1.results[c]["y_dir"] for c in range(8)]
    for c in range(4, 8):
        y_dirs[c] = y_dirs[c][:, ::-1]

    gate_w = inputs["gate_w"].astype(np.float32)
    gwA = _col_tiles(gate_w[:D_MODEL]).reshape(128, NKT * NKT * 128).astype(NB)
    gwB = _col_tiles(gate_w[D_MODEL:]).reshape(128, NKT * NKT * 128).astype(NB)
    gb = _tile_vec(inputs["gate_b"], NKT)
    pw = _col_tiles(inputs["proj_w"].astype(np.float32)).reshape(
        128, NKT * NKT * 128
    ).astype(NB)
    pb = _tile_vec(inputs["proj_b"], NKT)

    def _ytile(y):
        return np.ascontiguousarray(
            y.reshape(NKT, 128, TH).transpose(1, 0, 2)
        ).reshape(128, NKT * TH)

    maps2 = []
    for core in range(8):
        b = core % 4
        half = core // 4
        sl = slice(half * TH, (half + 1) * TH)
        maps2.append(
            dict(
                yA=_ytile(np.ascontiguousarray(y_dirs[b][:, sl])),
                yB=_ytile(np.ascontiguousarray(y_dirs[4 + b][:, sl])),
                gwA=gwA, gwB=gwB, gb=gb, pw=pw, pb=pb,
            )
        )
    global _last_maps2
    _last_maps2 = maps2
    res2 = run_bass_kernel_spmd(nc2, maps2, list(range(8)))

    out = np.empty((BATCH, SEQ, D_MODEL), np.float32)
    for core in range(8):
        b = core % 4
        half = core // 4
        o = res2.results[core]["out"]
        out[b, half * TH : (half + 1) * TH, :] = o.T
    return out
